# revision 43
# baseline (speedup 1.0000x reference)
"""Fused multi-head cross-attention with relation branch, sharded over 8 NeuronCores.

Sharding: data-parallel over batch (4) x tensor-parallel over head halves (2).
Core c handles batch c//2, heads [8*(c%2), 8*(c%2)+8). Each core computes its
partial output projection; the host sums the two partials per batch and adds bo.

Device data flow (per core):
  - q/k/rk projections emitted transposed: qT/kT/rkT [512 local dims, 1024 L]
    (4 chunks of 128 dims = head pairs (2dc, 2dc+1) at partitions 0-63/64-127)
  - v/rv projections emitted natural: [1024 LK, 512 dims], stored per lk-chunk
    with 64 ones-columns appended per head ([v_h | 1*64] of width 128) so the
    PV matmul emits the softmax denominator REPLICATED on PSUM rows 64-127
    (matmul time depends only on stream columns, so the wider stationary is
    free).
  - scores computed transposed sT[lk, lq] = kT.T @ qT per head, two heads
    row-packed on the PE array (K=64 each at array rows 0-63 / 64-127).
  - exp + mask + 1/sqrt(dk) fused into one ACT op per score tile:
    p = exp(s*scale + bias[lk]) with bias = 0 / -1e9 from the key mask.
  - x_att^T accumulated in PSUM over lk chunks: [v_h|1s].T @ p -> [128, lq].
  - normalize, mid iterations (latency hidden, engines cheap): denominators
    batch-reciprocated on 128 DVE lanes via an SBUF reshape DMA + DRAM-bounce
    partition broadcast; runs in bf16. The upper xf half is written by a
    cross-half DVE add (no partition-shift DMA).
  - normalize, last two iterations (latency critical): approx-fast DVE
    reciprocal straight on the replicated PSUM denominator rows, multiply,
    add - no DMAs in the chain, so the final output projection is gated only
    by ~3us of DVE work and the PE stays at its fast p-state.
  - output projection yT = WoT.T @ x_final accumulated over 4 dim chunks;
    yT ships as bf16 and the host sums the two tensor-parallel partials in
    fp32. The lq-half-1 chains for ot0-3 pre-accumulate dim chunks 0-2 on
    idle spool banks; ot4-7 finish in two pairs so each pair's PSUM copies
    and stores overlap the next pair's matmuls.
  - drain: output copies alternate ACT/DVE, the ysb ring is 8 deep so no
    copy ever waits a store completion, and all final stores issue on the
    sync queue (the gpsimd queue stays empty at the end - its completion
    drain is slow).
  - startup: x chunks stream on the sync queue, w chunks on gpsimd, consts
    on scalar; the transposed projections run staggered dc chains (chain
    dc's k4-7 interleaved with chain dc+1's k0-3) so chunk consumption
    never outruns DMA supply.

Matmul stage dtypes are configurable (bf16 for fast weight loads vs f32r for
accuracy); PSUM accumulation is always fp32.
"""

import math

import numpy as np

B, LQ, LK, D, H = 4, 1024, 1024, 1024, 16
DK = D // H
SCALE = 1.0 / math.sqrt(DK)
N_CORES = 8
HD = D // 2  # local dims per core (8 heads * 64)
# Keys are compacted host-side: only unmasked keys are shipped (padded to LKP
# with dummy rows whose mask bias is -1e9, so exp()=0 -> exact same math).
# mask ~ Bernoulli(1/2) over 1024 keys => valid ~ N(512, 16); 640 is +8 sigma.
LKP = 640
NM = LKP // 128  # lk chunks

_CACHE = {}

# Precision config: which matmul stages run bf16 (fast LDW) vs f32r (accurate).
CFG = {
    "score_bf16": True,  # query/key inputs, wq/wk, qT/kT/rkT score operands
    "v_bf16": True,      # value/rela inputs, wv/wrv/wrk, v_sb/rv_sb, p tiles
    "out_bf16": True,    # woT and x_final
}


def _build_program(lkp=LKP):
    import concourse.bacc as bacc
    import concourse.mybir as mybir
    import concourse.tile as tile

    LKP = lkp
    NM = LKP // 128

    f32 = mybir.dt.float32
    f32r = mybir.dt.float32r
    bf16 = mybir.dt.bfloat16
    f8 = mybir.dt.float8e4
    Exp = mybir.ActivationFunctionType.Exp
    Copy = mybir.ActivationFunctionType.Copy
    Add = mybir.AluOpType.add
    Mult = mybir.AluOpType.mult

    sdt = bf16 if CFG["score_bf16"] else f32
    vdt = bf16 if CFG["v_bf16"] else f32
    odt = bf16 if CFG["out_bf16"] else f32

    def mm_in(ap):
        """Matmul-operand view: f32 tiles feed the PE as f32r."""
        return ap.bitcast(f32r) if ap.dtype == f32 else ap

    def prod(ap):
        """Producer-output view for matmul-consumed tiles (verifier wants the
        producer to emit f32r when the consumer reads f32r)."""
        return ap.bitcast(f32r) if ap.dtype == f32 else ap

    nc = bacc.Bacc(
        "TRN2",
        target_bir_lowering=False,
        debug=False,
        enable_asserts=False,
        num_devices=N_CORES,
    )

    # DRAM I/O (per-core shapes; host shards/pre-transposes/casts).
    xqT = nc.dram_tensor("xqT", [D, LQ], sdt, kind="ExternalInput").ap()
    xkT = nc.dram_tensor("xkT", [D, LKP], sdt, kind="ExternalInput").ap()
    wqT = nc.dram_tensor("wqT", [D, HD], sdt, kind="ExternalInput").ap()
    wkT = nc.dram_tensor("wkT", [D, HD], sdt, kind="ExternalInput").ap()
    wrkT = nc.dram_tensor("wrkT", [D, HD], vdt, kind="ExternalInput").ap()
    xrT = nc.dram_tensor("xrT", [D, LKP], vdt, kind="ExternalInput").ap()
    xvT = nc.dram_tensor("xvT", [D, LKP], vdt, kind="ExternalInput").ap()
    wvT = nc.dram_tensor("wvT", [D, HD], vdt, kind="ExternalInput").ap()
    wrvT = nc.dram_tensor("wrvT", [D, HD], vdt, kind="ExternalInput").ap()
    woT = nc.dram_tensor("woT", [HD, D], odt, kind="ExternalInput").ap()
    bq_pc = nc.dram_tensor("bq_pc", [128, 4], f32, kind="ExternalInput").ap()
    bk_pc = nc.dram_tensor("bk_pc", [128, 4], f32, kind="ExternalInput").ap()
    brk_pc = nc.dram_tensor("brk_pc", [128, 4], f32, kind="ExternalInput").ap()
    bv_bc = nc.dram_tensor("bv_bc", [128, HD], f32, kind="ExternalInput").ap()
    brv_bc = nc.dram_tensor("brv_bc", [128, HD], f32, kind="ExternalInput").ap()
    maskb = nc.dram_tensor("maskb", [128, NM], f32, kind="ExternalInput").ap()
    yT = nc.dram_tensor("yT", [D, LQ], odt, kind="ExternalOutput").ap()
    scr2 = nc.dram_tensor("scr2", [8, 2048], vdt, kind="Internal").ap()

    def r(ap):
        return ap.bitcast(f32r)

    with tile.TileContext(nc) as tc:
        from contextlib import ExitStack

        with ExitStack() as ctx:
            # The softmax normalize chain runs in bf16 for 2x DVE throughput;
            # the ~0.4% relative error it adds is well inside the tolerance.
            ctx.enter_context(
                nc.allow_low_precision(reason="bf16 softmax normalize chain")
            )
            # Persistent SBUF tensors. v/rv carry 64 ones-columns per (m, h)
            # block so the PV matmul emits the softmax denominator REPLICATED
            # on PSUM partitions 64-127 (out rows 0-63 = x, 64-127 = Z).
            persist = ctx.enter_context(tc.tile_pool(name="persist", bufs=1))
            qT_sb = persist.tile([128, 4 * LQ], sdt, tag="qT")
            kT_sb = persist.tile([128, 4 * LKP], sdt, tag="kT")
            rkT_sb = persist.tile([128, 4 * LKP], sdt, tag="rkT")
            v_sb = persist.tile([128, NM * 8 * 128], vdt, tag="v")
            rv_sb = persist.tile([128, NM * 8 * 128], vdt, tag="rv")
            xf_sb = persist.tile([128, 4 * LQ], odt, tag="xf")
            maskb_sb = persist.tile([128, NM], f32, tag="maskb")
            bq_sb = persist.tile([128, 4], f32, tag="bq")
            bk_sb = persist.tile([128, 4], f32, tag="bk")
            brk_sb = persist.tile([128, 4], f32, tag="brk")
            bv_sb = persist.tile([128, HD], f32, tag="bv")
            brv_sb = persist.tile([128, HD], f32, tag="brv")

            # Consts on the scalar queue: the sync queue streams x chunks and
            # the gpsimd queue streams w chunks from the first instruction.
            nc.scalar.dma_start(out=maskb_sb[:], in_=maskb)
            nc.scalar.dma_start(out=bq_sb[:], in_=bq_pc)
            nc.scalar.dma_start(out=bk_sb[:], in_=bk_pc)
            nc.scalar.dma_start(out=brk_sb[:], in_=brk_pc)
            nc.scalar.dma_start(out=bv_sb[:], in_=bv_bc)
            nc.scalar.dma_start(out=brv_sb[:], in_=brv_bc)

            assert vdt == bf16 and sdt == bf16 and odt == bf16
            v4 = v_sb[:].rearrange("p (m h c) -> p m h c", m=NM, h=8, c=128)
            rv4 = rv_sb[:].rearrange("p (m h c) -> p m h c", m=NM, h=8, c=128)
            nc.vector.memset(v4[:, :, :, 64:128], 1.0)
            nc.vector.memset(rv4[:, :, :, 64:128], 1.0)

            # Score/exp pools opened BEFORE the projection pools so their PSUM
            # banks are disjoint from the projection psum banks (no false WAR:
            # scores may start as soon as q/k/rk chunks are ready).
            spool = ctx.enter_context(tc.tile_pool(name="spool", bufs=2, space="PSUM"))
            ppool = ctx.enter_context(tc.tile_pool(name="ppool", bufs=34))

            p_tiles = {}

            def emit_scores(lqh, dcs=(0, 1, 2, 3), brs=(0, 1)):
                for dc in dcs:
                    qsl = slice(1024 * dc + 512 * lqh, 1024 * dc + 512 * lqh + 512)
                    for m in range(NM):
                        ksl = slice(LKP * dc + 128 * m, LKP * dc + 128 * m + 128)
                        for br, kt in ((0, kT_sb), (1, rkT_sb)):
                            if br not in brs:
                                continue
                            s = spool.tile([128, 1024], f32, tag="spool", name="s")
                            nc.tensor.matmul(
                                s[:, 0:512], mm_in(kt[0:64, ksl]), mm_in(qT_sb[0:64, qsl])
                            )
                            nc.tensor.matmul(
                                s[:, 512:1024],
                                mm_in(kt[64:128, ksl]),
                                mm_in(qT_sb[64:128, qsl]),
                            )
                            p = ppool.tile([128, 1024], vdt, tag="ppool", name="p")
                            nc.scalar.activation(
                                prod(p[:]),
                                s[:],
                                Exp,
                                bias=maskb_sb[:, m : m + 1],
                                scale=SCALE,
                            )
                            p_tiles[(lqh, dc, m, br)] = p


            # ---------------- Phase 1: projections ----------------
            with ExitStack() as ph1:
                inp = ph1.enter_context(tc.tile_pool(name="inp", bufs=32))
                wch_pool = ph1.enter_context(tc.tile_pool(name="wch", bufs=24))
                ppsum = ph1.enter_context(
                    tc.tile_pool(name="ppsum", bufs=2, space="PSUM")
                )

                # Transposed projections: out chunk dc = lhsT(W block).T @ x_chunk.
                # The qk-branch scores are emitted between the k and rk
                # projections - they only need q+k, so the ACT exp stream
                # starts a full projection earlier.
                rk_xch = None
                proj_specs = [
                    ("k", xkT, wkT, bk_sb, kT_sb, sdt, LKP),
                    ("rk", xrT, wrkT, brk_sb, rkT_sb, vdt, LKP),
                ]
                for name, xt, wt, b_sb, out_sb, dt_, LL in proj_specs:
                    nsl = [slice(a, min(a + 512, LL)) for a in range(0, LL, 512)]
                    xch = []
                    wch = []
                    for k in range(8):
                        t = inp.tile([128, LL], dt_, tag="inp", name=f"x{name}{k}")
                        nc.sync.dma_start(
                            out=prod(t[:]), in_=mm_in(xt[128 * k : 128 * k + 128, :])
                        )
                        xch.append(t)
                        w = wch_pool.tile([128, HD], dt_, tag="wch", name=f"w{name}{k}")
                        nc.gpsimd.dma_start(
                            out=prod(w[:]), in_=mm_in(wt[128 * k : 128 * k + 128, :])
                        )
                        wch.append(w)
                    if name == "rk":
                        rk_xch = xch
                    # Staggered dc chains: chain dc's k4-7 matmuls interleave
                    # with chain dc+1's k0-3, so each input chunk is consumed
                    # at half rate and the PE never outruns the DMA supply at
                    # kernel start; bias-adds overlap the next stage.
                    def pmm(ps, dc, k):
                        for sl in nsl:
                            nc.tensor.matmul(
                                ps[:, sl],
                                mm_in(wch[k][:, 128 * dc : 128 * dc + 128]),
                                mm_in(xch[k][:, sl]),
                                start=(k == 0),
                                stop=(k == 7),
                            )

                    chains = [None] * 5
                    chains[0] = ppsum.tile(
                        [128, LL], f32, tag="ppsum", name=f"pp{name}0"
                    )
                    for k in range(4):
                        pmm(chains[0], 0, k)
                    for j in range(4):
                        if j + 1 < 4:
                            chains[j + 1] = ppsum.tile(
                                [128, LL], f32, tag="ppsum", name=f"pp{name}{j + 1}"
                            )
                        for k in range(4):
                            pmm(chains[j], j, 4 + k)
                            if j + 1 < 4:
                                pmm(chains[j + 1], j + 1, k)
                        nc.vector.tensor_scalar(
                            out=prod(out_sb[:, LL * j : LL * j + LL]),
                            in0=chains[j][:],
                            scalar1=b_sb[:, j : j + 1],
                            scalar2=None,
                            op0=Add,
                        )

                # q projection AFTER k/rk, one dc chain at a time with that
                # dc's lq-half-0 scores emitted immediately after: the first
                # exp fires right after k+rk+one q chain instead of after all
                # three projections (~20us earlier), and the k/rk PE time
                # covers the xq DMA supply so the chains never starve.
                xqch = []
                wqch = []
                for k in range(8):
                    t = inp.tile([128, LQ], sdt, tag="inp", name=f"xq{k}")
                    nc.sync.dma_start(
                        out=t[:], in_=xqT[128 * k : 128 * k + 128, :]
                    )
                    xqch.append(t)
                    w = wch_pool.tile([128, HD], sdt, tag="wch", name=f"wq{k}")
                    nc.gpsimd.dma_start(
                        out=w[:], in_=wqT[128 * k : 128 * k + 128, :]
                    )
                    wqch.append(w)
                for dc in range(4):
                    ps = ppsum.tile([128, LQ], f32, tag="ppsum", name=f"ppq{dc}")
                    for k in range(8):
                        for sl in (slice(0, 512), slice(512, 1024)):
                            nc.tensor.matmul(
                                ps[:, sl],
                                wqch[k][:, 128 * dc : 128 * dc + 128],
                                xqch[k][:, sl],
                                start=(k == 0),
                                stop=(k == 7),
                            )
                    nc.vector.tensor_scalar(
                        out=qT_sb[:, LQ * dc : LQ * dc + LQ],
                        in0=ps[:],
                        scalar1=bq_sb[:, dc : dc + 1],
                        scalar2=None,
                        op0=Add,
                    )
                    emit_scores(0, [dc])

                # Natural-orientation projections for v / rv. rv reuses rk's
                # input tiles (same weak_rela data, still resident in the inp
                # ring), and the second lq-half scores are split around rv so
                # the ACT engine never starves.
                def proj_N(name, xt, wt, b_sb, out4, xch_reuse):
                    if xch_reuse is not None:
                        xch = xch_reuse
                    else:
                        xch = []
                        for k in range(8):
                            t = inp.tile(
                                [128, LKP], vdt, tag="inp", name=f"x{name}{k}"
                            )
                            nc.sync.dma_start(
                                out=prod(t[:]),
                                in_=mm_in(xt[128 * k : 128 * k + 128, :]),
                            )
                            xch.append(t)
                    wch = []
                    for k in range(8):
                        w = wch_pool.tile([128, HD], vdt, tag="wch", name=f"w{name}{k}")
                        nc.gpsimd.dma_start(
                            out=prod(w[:]), in_=mm_in(wt[128 * k : 128 * k + 128, :])
                        )
                        wch.append(w)
                    for m in range(NM):
                        ps = ppsum.tile([128, 512], f32, tag="ppsum")
                        for k in range(8):
                            nc.tensor.matmul(
                                ps[:],
                                mm_in(xch[k][:, 128 * m : 128 * m + 128]),
                                mm_in(wch[k][:]),
                                start=(k == 0),
                                stop=(k == 7),
                            )
                        nc.vector.tensor_tensor(
                            out=prod(out4[:, m, :, 0:64]),
                            in0=ps[:].rearrange("p (h c) -> p h c", h=8, c=64),
                            in1=b_sb[:].rearrange("p (h c) -> p h c", h=8, c=64),
                            op=Add,
                        )

                proj_N("v", xvT, wvT, bv_sb, v4, None)
                emit_scores(1, [0, 1])
                proj_N("rv", xrT, wrvT, brv_sb, rv4, rk_xch)
                emit_scores(1, [2, 3])

            # -------- Phase B: PV accumulation, normalize, output projection ----
            with ExitStack() as ph2:
                xpool = ph2.enter_context(
                    tc.tile_pool(name="xpool", bufs=4, space="PSUM")
                )
                # One merged SBUF pool for all phase-2 scratch (tags keep
                # independent rings) - TileContext exit emits per-pool
                # per-engine semaphore teardown chains, so fewer pools means
                # a shorter serial teardown.
                p2sb = ph2.enter_context(tc.tile_pool(name="p2sb", bufs=8))
                xsb = sgp = bcp = rbp = wop = ysb = p2sb

                woch = []
                for dc in range(4):
                    w = wop.tile([128, 1024], odt, tag="wop", name=f"wo{dc}")
                    nc.sync.dma_start(
                        out=prod(w[:]), in_=mm_in(woT[128 * dc : 128 * dc + 128, :])
                    )
                    woch.append(w)

                def outproj_pair(lqh, pair, act=False):
                    # Two ot chains dc-outer (consecutive matmuls alternate
                    # banks), drained the moment they stop. Early pairs copy
                    # on DVE (ACT is streaming exps); late pairs copy on ACT
                    # (DVE is running the tail normalize).
                    ps2 = {
                        ot: xpool.tile(
                            [128, 512], f32, tag="xpool", name=f"psy{ot}"
                        )
                        for ot in pair
                    }
                    for dc in range(4):
                        for ot in pair:
                            nc.tensor.matmul(
                                ps2[ot][:],
                                mm_in(woch[dc][:, 128 * ot : 128 * ot + 128]),
                                mm_in(
                                    xf_sb[
                                        :,
                                        1024 * dc
                                        + 512 * lqh : 1024 * dc
                                        + 512 * lqh
                                        + 512,
                                    ]
                                ),
                                start=(dc == 0),
                                stop=(dc == 3),
                            )
                    for ot in pair:
                        y = ysb.tile([128, 512], odt, tag="ysb")
                        if act:
                            nc.scalar.activation(y[:], ps2[ot][:], Copy)
                        else:
                            nc.vector.tensor_copy(out=y[:], in_=ps2[ot][:])
                        (nc.sync if ot % 2 == 0 else nc.gpsimd).dma_start(
                            out=yT[
                                128 * ot : 128 * ot + 128,
                                512 * lqh : 512 * lqh + 512,
                            ],
                            in_=y[:],
                        )

                def emit_it(lqh, dc, tail=False):
                    xacc = {}
                    accs = [(0, 0), (1, 0), (0, 1), (1, 1)]
                    for br, hs in accs:
                        xacc[(br, hs)] = xpool.tile(
                            [128, 512], f32, tag="xpool", name=f"xacc{br}{hs}"
                        )
                    vvs = {0: v4, 1: rv4}

                    def pv_step(br, hs, m):
                        pt = p_tiles[(lqh, dc, m, br)]
                        nc.tensor.matmul(
                            xacc[(br, hs)][:],
                            mm_in(vvs[br][:, m, 2 * dc + hs, :]),
                            mm_in(pt[:, 512 * hs : 512 * hs + 512]),
                            start=(m == 0),
                            stop=(m == NM - 1),
                        )

                    xf_slice = slice(
                        1024 * dc + 512 * lqh, 1024 * dc + 512 * lqh + 512
                    )
                    if tail:
                        # hs=0 chains first so their normalize overlaps the
                        # hs=1 PV matmuls; consecutive matmuls still alternate
                        # banks within each pair.
                        for hs in range(2):
                            for m in range(NM):
                                for br in range(2):
                                    pv_step(br, hs, m)
                            ts = {}
                            for br in range(2):
                                zt = rbp.tile([64, 512], f32, tag="zt")
                                nc.scalar.activation(
                                    zt[:], xacc[(br, hs)][64:128, :], Copy
                                )
                                rb = rbp.tile([64, 512], f32, tag="rbp")
                                nc.vector.reciprocal_approx_fast(
                                    out=rb[:], in_=zt[:]
                                )
                                tt = xsb.tile([65, 512], vdt, tag="xsb")
                                nc.vector.tensor_tensor(
                                    out=prod(tt[0:64, :]),
                                    in0=xacc[(br, hs)][0:64, :],
                                    in1=rb[:],
                                    op=Mult,
                                )
                                ts[br] = tt
                            nc.vector.tensor_tensor(
                                out=prod(xf_sb[64 * hs : 64 * hs + 64, xf_slice]),
                                in0=ts[0][0:64, :],
                                in1=ts[1][0:64, :],
                                op=Add,
                            )
                        return
                    for m in range(NM):
                        for br, hs in accs:
                            pv_step(br, hs, m)
                    # Copy x rows + denominator row 64 to SBUF (frees the PSUM
                    # banks early), packed so the 4 denominator rows ship to
                    # DRAM in a single DMA.
                    xs_all = xsb.tile([65, 4 * 512], vdt, tag="xsall", bufs=3)
                    xs = {}
                    for j, (br, hs) in enumerate(accs):
                        sl = xs_all[:, 512 * j : 512 * j + 512]
                        nc.vector.tensor_copy(out=sl, in_=xacc[(br, hs)][0:65, :])
                        xs[(br, hs)] = sl
                    # Batch-reciprocate the 4 denominator rows: direct
                    # SBUF->SBUF reshape onto 128 partitions, reciprocal,
                    # one DRAM bounce for the partition broadcast.
                    it = 2 * dc + lqh
                    sg = sgp.tile([128, 16], vdt, tag="sgp")
                    nc.sync.dma_start(out=sg[:], in_=xs_all[64:65, :])
                    nc.vector.reciprocal(sg[:], sg[:])
                    nc.sync.dma_start(out=scr2[it, :], in_=sg[:])
                    for hs in range(2):
                        jv, jr = 2 * hs, 2 * hs + 1
                        bcv = bcp.tile([64, 512], vdt, tag="bcp", name="bcv")
                        nc.gpsimd.dma_start(
                            out=bcv[:],
                            in_=scr2[it : it + 1, 512 * jv : 512 * jv + 512]
                            .partition_broadcast(64)[:, 0, :],
                        )
                        bcr = bcp.tile([64, 512], vdt, tag="bcp", name="bcr")
                        nc.gpsimd.dma_start(
                            out=bcr[:],
                            in_=scr2[it : it + 1, 512 * jr : 512 * jr + 512]
                            .partition_broadcast(64)[:, 0, :],
                        )
                        t1 = xsb.tile([65, 512], vdt, tag="xsb")
                        nc.vector.tensor_tensor(
                            out=t1[0:64, :],
                            in0=xs[(0, hs)][0:64, :],
                            in1=bcv[:],
                            op=Mult,
                        )
                        t2 = xsb.tile([65, 512], vdt, tag="xsb")
                        nc.vector.tensor_tensor(
                            out=t2[0:64, :],
                            in0=xs[(1, hs)][0:64, :],
                            in1=bcr[:],
                            op=Mult,
                        )
                        nc.vector.tensor_tensor(
                            out=prod(xf_sb[64 * hs : 64 * hs + 64, xf_slice]),
                            in0=t1[0:64, :],
                            in1=t2[0:64, :],
                            op=Add,
                        )
                emit_it(0, 0)
                emit_it(0, 1)
                emit_it(0, 2)
                emit_it(0, 3)
                emit_it(1, 0)
                emit_it(1, 1, tail=True)
                # lq-half-0 output projection, wide: all 8 ot accumulators
                # live at once (4 xpool banks + 2 spool tiles split in half),
                # dc-outer so consecutive matmuls alternate banks.
                pssw = []
                for i in range(4):
                    pssw.append(
                        xpool.tile([128, 512], f32, tag="xpool", name=f"psw{i}")
                    )
                for i in range(2):
                    w2 = spool.tile([128, 1024], f32, tag="spool", name=f"psw2{i}")
                    pssw.append(w2[:, 0:512])
                    pssw.append(w2[:, 512:1024])
                for dc in range(4):
                    for ot in range(8):
                        nc.tensor.matmul(
                            pssw[ot],
                            mm_in(woch[dc][:, 128 * ot : 128 * ot + 128]),
                            mm_in(xf_sb[:, 1024 * dc : 1024 * dc + 512]),
                            start=(dc == 0),
                            stop=(dc == 3),
                        )
                for ot in range(8):
                    y = ysb.tile([128, 512], odt, tag="ysb")
                    if ot % 2 == 0:
                        nc.scalar.activation(y[:], pssw[ot], Copy)
                    else:
                        nc.vector.tensor_copy(out=y[:], in_=pssw[ot])
                    (nc.sync if ot % 2 == 0 else nc.gpsimd).dma_start(
                        out=yT[128 * ot : 128 * ot + 128, 0:512],
                        in_=y[:],
                    )
                emit_it(1, 2, tail=True)
                # Pre-emit ot0-3 of the lq-half-1 output projection over the
                # first three dim chunks on the spool banks (free once scores
                # are done), so those 12 matmuls overlap the final dance and
                # only ot0-3's dc3 plus ot4-7 trail the last normalize.
                pss = []
                for i in range(2):
                    w2 = spool.tile([128, 1024], f32, tag="spool", name=f"psw2{i}")
                    pss.append(w2[:, 0:512])
                    pss.append(w2[:, 512:1024])
                for dcc in range(3):
                    for ot in range(4):
                        nc.tensor.matmul(
                            pss[ot],
                            mm_in(woch[dcc][:, 128 * ot : 128 * ot + 128]),
                            mm_in(xf_sb[:, 1024 * dcc + 512 : 1024 * dcc + 1024]),
                            start=(dcc == 0),
                            stop=False,
                        )
                emit_it(1, 3, tail=True)

                # Drain an ot chain the moment it stops: copy alternates
                # ACT/DVE, stores go on the sync/scalar queues (gpsimd stays
                # empty at the end so its completion drain is off the
                # critical path).
                def drain_ot(ot):
                    y = ysb.tile([128, 512], odt, tag="ysb")
                    if ot % 2 == 0:
                        nc.scalar.activation(y[:], pss[ot], Copy)
                    else:
                        nc.vector.tensor_copy(out=y[:], in_=pss[ot])
                    nc.sync.dma_start(
                        out=yT[128 * ot : 128 * ot + 128, 512:1024], in_=y[:]
                    )

                for ot in range(4):
                    nc.tensor.matmul(
                        pss[ot],
                        mm_in(woch[3][:, 128 * ot : 128 * ot + 128]),
                        mm_in(xf_sb[:, 1024 * 3 + 512 : 1024 * 3 + 1024]),
                        start=False,
                        stop=True,
                    )
                    drain_ot(ot)
                for i in range(4):
                    pss.append(
                        xpool.tile([128, 512], f32, tag="xpool", name=f"psw{i}")
                    )
                # ot4-7 in two pairs: the first pair's copies and stores
                # overlap the second pair's matmuls.
                for pair in ((4, 5), (6, 7)):
                    for dcc in range(4):
                        for ot in pair:
                            nc.tensor.matmul(
                                pss[ot],
                                mm_in(woch[dcc][:, 128 * ot : 128 * ot + 128]),
                                mm_in(
                                    xf_sb[:, 1024 * dcc + 512 : 1024 * dcc + 1024]
                                ),
                                start=(dcc == 0),
                                stop=(dcc == 3),
                            )
                    for ot in pair:
                        drain_ot(ot)

    nc.compile()
    return nc


def _get_program(lkp=LKP):
    if lkp not in _CACHE:
        _CACHE[lkp] = _build_program(lkp)
    return _CACHE[lkp]


def _f8c(arr):
    """[1024, L] fp32 -> [128, 8*L] fp8e4m3 chunked: [p, k, n] = arr[128k+p, n]."""
    import ml_dtypes

    a = np.ascontiguousarray(arr, dtype=np.float32)
    L = a.shape[1]
    return np.ascontiguousarray(
        a.reshape(8, 128, L)
        .transpose(1, 0, 2)
        .reshape(128, 8 * L)
        .astype(ml_dtypes.float8_e4m3)
    )


def _cast(arr, bf16_flag):
    if bf16_flag:
        import ml_dtypes

        return np.ascontiguousarray(arr.astype(ml_dtypes.bfloat16))
    return np.ascontiguousarray(arr.astype(np.float32))


def _shard_inputs(inputs, lkp=LKP):
    q = np.ascontiguousarray(inputs["query"], dtype=np.float32)
    k = np.ascontiguousarray(inputs["key"], dtype=np.float32)
    v = np.ascontiguousarray(inputs["value"], dtype=np.float32)
    wr = np.ascontiguousarray(inputs["weak_rela"], dtype=np.float32)
    mask = np.asarray(inputs["mask"])
    sb, vb, ob = CFG["score_bf16"], CFG["v_bf16"], CFG["out_bf16"]

    in_maps = []
    for c in range(N_CORES):
        b, hh = divmod(c, 2)
        hsl = slice(HD * hh, HD * hh + HD)
        idx = np.nonzero(mask[b, 0])[0]
        nv = len(idx)
        assert nv <= lkp
        pidx = np.concatenate([idx, np.zeros(lkp - nv, dtype=idx.dtype)])
        bias = np.full(lkp, -1.0e9, np.float32)
        bias[:nv] = 0.0
        mb = np.ascontiguousarray(bias.reshape(lkp // 128, 128).T)
        kc, vc, wrc = k[b][pidx], v[b][pidx], wr[b][pidx]
        m = {
            "xqT": _cast(q[b].T, sb),
            "xkT": _cast(kc.T, sb),
            "xrT": _cast(wrc.T, vb),
            "xvT": _cast(vc.T, vb),
            "wqT": _cast(np.asarray(inputs["Wq"])[hsl, :].T, sb),
            "wkT": _cast(np.asarray(inputs["Wk"])[hsl, :].T, sb),
            "wrkT": _cast(np.asarray(inputs["Wrk"])[hsl, :].T, vb),
            "wvT": _cast(np.asarray(inputs["Wv"])[hsl, :].T, vb),
            "wrvT": _cast(np.asarray(inputs["Wrv"])[hsl, :].T, vb),
            "woT": _cast(np.asarray(inputs["Wo"])[:, hsl].T, ob),
            "bq_pc": np.asarray(inputs["bq"][hsl])
            .reshape(4, 128)
            .T.astype(np.float32),
            "bk_pc": np.asarray(inputs["bk"][hsl])
            .reshape(4, 128)
            .T.astype(np.float32),
            "brk_pc": np.asarray(inputs["brk"][hsl])
            .reshape(4, 128)
            .T.astype(np.float32),
            "bv_bc": np.broadcast_to(inputs["bv"][hsl], (128, HD)).astype(np.float32),
            "brv_bc": np.broadcast_to(inputs["brv"][hsl], (128, HD)).astype(
                np.float32
            ),
            "maskb": mb,
        }
        in_maps.append({k2: np.ascontiguousarray(v2) for k2, v2 in m.items()})
    return in_maps


def run_on_hw(inputs, trace=False, **kw):
    from concourse.bass_utils import run_bass_kernel_spmd

    mask = np.asarray(inputs["mask"])
    max_valid = max(int(mask[b, 0].sum()) for b in range(B))
    lkp = max(LKP, ((max_valid + 127) // 128) * 128)
    nc = _get_program(lkp)
    in_maps = _shard_inputs(inputs, lkp)
    res = run_bass_kernel_spmd(
        nc, in_maps, core_ids=list(range(N_CORES)), trace=trace, **kw
    )
    bo = np.asarray(inputs["bo"], dtype=np.float32)
    outs = []
    for b in range(B):
        yt = res.results[2 * b]["yT"].astype(np.float32) + res.results[
            2 * b + 1
        ]["yT"].astype(np.float32)
        outs.append(yt.T + bo)
    out = np.stack(outs).astype(np.float32)
    return out, res


def kernel(**inputs):
    out, _ = run_on_hw(inputs)
    return out



# revision 45
# speedup vs baseline: 1.0231x; 1.0231x over previous
"""Fused multi-head cross-attention with relation branch, sharded over 8 NeuronCores.

Sharding: data-parallel over batch (4) x tensor-parallel over head halves (2).
Core c handles batch c//2, heads [8*(c%2), 8*(c%2)+8). Each core computes its
partial output projection; the host sums the two partials per batch and adds bo.

Device data flow (per core):
  - q/k/rk projections emitted transposed: qT/kT/rkT [512 local dims, 1024 L]
    (4 chunks of 128 dims = head pairs (2dc, 2dc+1) at partitions 0-63/64-127)
  - v/rv projections emitted natural: [1024 LK, 512 dims], stored per lk-chunk
    with 64 ones-columns appended per head ([v_h | 1*64] of width 128) so the
    PV matmul emits the softmax denominator REPLICATED on PSUM rows 64-127
    (matmul time depends only on stream columns, so the wider stationary is
    free).
  - scores computed transposed sT[lk, lq] = kT.T @ qT per head, two heads
    row-packed on the PE array (K=64 each at array rows 0-63 / 64-127).
  - exp + mask + 1/sqrt(dk) fused into one ACT op per score tile:
    p = exp(s*scale + bias[lk]) with bias = 0 / -1e9 from the key mask.
  - x_att^T accumulated in PSUM over lk chunks: [v_h|1s].T @ p -> [128, lq].
  - normalize, mid iterations (latency hidden, engines cheap): denominators
    batch-reciprocated on 128 DVE lanes via an SBUF reshape DMA + DRAM-bounce
    partition broadcast; runs in bf16. The upper xf half is written by a
    cross-half DVE add (no partition-shift DMA).
  - normalize, last two iterations (latency critical): approx-fast DVE
    reciprocal straight on the replicated PSUM denominator rows, multiply,
    add - no DMAs in the chain, so the final output projection is gated only
    by ~3us of DVE work and the PE stays at its fast p-state.
  - output projection yT = WoT.T @ x_final accumulated over 4 dim chunks;
    yT ships as bf16 and the host sums the two tensor-parallel partials in
    fp32. The lq-half-1 chains for ot0-3 pre-accumulate dim chunks 0-2 on
    idle spool banks; ot4-7 finish in two pairs so each pair's PSUM copies
    and stores overlap the next pair's matmuls.
  - drain: output copies alternate ACT/DVE, the ysb ring is 8 deep so no
    copy ever waits a store completion, and all final stores issue on the
    sync queue (the gpsimd queue stays empty at the end - its completion
    drain is slow).
  - startup: x chunks stream on the sync queue, w chunks on gpsimd, consts
    on scalar; the transposed projections run staggered dc chains (chain
    dc's k4-7 interleaved with chain dc+1's k0-3) so chunk consumption
    never outruns DMA supply.

Matmul stage dtypes are configurable (bf16 for fast weight loads vs f32r for
accuracy); PSUM accumulation is always fp32.
"""

import math

import numpy as np

B, LQ, LK, D, H = 4, 1024, 1024, 1024, 16
DK = D // H
SCALE = 1.0 / math.sqrt(DK)
N_CORES = 8
HD = D // 2  # local dims per core (8 heads * 64)
# Keys are compacted host-side: only unmasked keys are shipped (padded to LKP
# with dummy rows whose mask bias is -1e9, so exp()=0 -> exact same math).
# mask ~ Bernoulli(1/2) over 1024 keys => valid ~ N(512, 16); 640 is +8 sigma.
LKP = 640
NM = LKP // 128  # lk chunks

_CACHE = {}

# Precision config: which matmul stages run bf16 (fast LDW) vs f32r (accurate).
CFG = {
    "score_bf16": True,  # query/key inputs, wq/wk, qT/kT/rkT score operands
    "v_bf16": True,      # value/rela inputs, wv/wrv/wrk, v_sb/rv_sb, p tiles
    "out_bf16": True,    # woT and x_final
}


def _build_program(lkp=LKP):
    import concourse.bacc as bacc
    import concourse.mybir as mybir
    import concourse.tile as tile

    LKP = lkp
    NM = LKP // 128

    f32 = mybir.dt.float32
    f32r = mybir.dt.float32r
    bf16 = mybir.dt.bfloat16
    f8 = mybir.dt.float8e4
    Exp = mybir.ActivationFunctionType.Exp
    Copy = mybir.ActivationFunctionType.Copy
    Add = mybir.AluOpType.add
    Mult = mybir.AluOpType.mult

    sdt = bf16 if CFG["score_bf16"] else f32
    vdt = bf16 if CFG["v_bf16"] else f32
    odt = bf16 if CFG["out_bf16"] else f32

    def mm_in(ap):
        """Matmul-operand view: f32 tiles feed the PE as f32r."""
        return ap.bitcast(f32r) if ap.dtype == f32 else ap

    def prod(ap):
        """Producer-output view for matmul-consumed tiles (verifier wants the
        producer to emit f32r when the consumer reads f32r)."""
        return ap.bitcast(f32r) if ap.dtype == f32 else ap

    nc = bacc.Bacc(
        "TRN2",
        target_bir_lowering=False,
        debug=False,
        enable_asserts=False,
        num_devices=N_CORES,
    )

    # DRAM I/O (per-core shapes; host shards/pre-transposes/casts).
    xqT = nc.dram_tensor("xqT", [D, LQ], sdt, kind="ExternalInput").ap()
    xkT = nc.dram_tensor("xkT", [D, LKP], sdt, kind="ExternalInput").ap()
    wqT = nc.dram_tensor("wqT", [D, HD], sdt, kind="ExternalInput").ap()
    wkT = nc.dram_tensor("wkT", [D, HD], sdt, kind="ExternalInput").ap()
    wrkT = nc.dram_tensor("wrkT", [D, HD], vdt, kind="ExternalInput").ap()
    xrT = nc.dram_tensor("xrT", [D, LKP], vdt, kind="ExternalInput").ap()
    xvT = nc.dram_tensor("xvT", [D, LKP], vdt, kind="ExternalInput").ap()
    wvT = nc.dram_tensor("wvT", [D, HD], vdt, kind="ExternalInput").ap()
    wrvT = nc.dram_tensor("wrvT", [D, HD], vdt, kind="ExternalInput").ap()
    woT = nc.dram_tensor("woT", [HD, D], odt, kind="ExternalInput").ap()
    bq_pc = nc.dram_tensor("bq_pc", [128, 4], f32, kind="ExternalInput").ap()
    bk_pc = nc.dram_tensor("bk_pc", [128, 4], f32, kind="ExternalInput").ap()
    brk_pc = nc.dram_tensor("brk_pc", [128, 4], f32, kind="ExternalInput").ap()
    bv_bc = nc.dram_tensor("bv_bc", [128, HD], f32, kind="ExternalInput").ap()
    brv_bc = nc.dram_tensor("brv_bc", [128, HD], f32, kind="ExternalInput").ap()
    maskb = nc.dram_tensor("maskb", [128, NM], f32, kind="ExternalInput").ap()
    yT = nc.dram_tensor("yT", [D, LQ], odt, kind="ExternalOutput").ap()
    scr2 = nc.dram_tensor("scr2", [8, 2048], vdt, kind="Internal").ap()

    def r(ap):
        return ap.bitcast(f32r)

    with tile.TileContext(nc) as tc:
        from contextlib import ExitStack

        with ExitStack() as ctx:
            # The softmax normalize chain runs in bf16 for 2x DVE throughput;
            # the ~0.4% relative error it adds is well inside the tolerance.
            ctx.enter_context(
                nc.allow_low_precision(reason="bf16 softmax normalize chain")
            )
            # Persistent SBUF tensors. v/rv carry 64 ones-columns per (m, h)
            # block so the PV matmul emits the softmax denominator REPLICATED
            # on PSUM partitions 64-127 (out rows 0-63 = x, 64-127 = Z).
            persist = ctx.enter_context(tc.tile_pool(name="persist", bufs=1))
            qT_sb = persist.tile([128, 4 * LQ], sdt, tag="qT")
            kT_sb = persist.tile([128, 4 * LKP], sdt, tag="kT")
            rkT_sb = persist.tile([128, 4 * LKP], sdt, tag="rkT")
            v_sb = persist.tile([128, NM * 8 * 128], vdt, tag="v")
            rv_sb = persist.tile([128, NM * 8 * 128], vdt, tag="rv")
            xf_sb = persist.tile([128, 4 * LQ], odt, tag="xf")
            maskb_sb = persist.tile([128, NM], f32, tag="maskb")
            bq_sb = persist.tile([128, 4], f32, tag="bq")
            bk_sb = persist.tile([128, 4], f32, tag="bk")
            brk_sb = persist.tile([128, 4], f32, tag="brk")
            bv_sb = persist.tile([128, HD], f32, tag="bv")
            brv_sb = persist.tile([128, HD], f32, tag="brv")

            # Consts on the scalar queue: the sync queue streams x chunks and
            # the gpsimd queue streams w chunks from the first instruction.
            nc.scalar.dma_start(out=maskb_sb[:], in_=maskb)
            nc.scalar.dma_start(out=bq_sb[:], in_=bq_pc)
            nc.scalar.dma_start(out=bk_sb[:], in_=bk_pc)
            nc.scalar.dma_start(out=brk_sb[:], in_=brk_pc)
            nc.scalar.dma_start(out=bv_sb[:], in_=bv_bc)
            nc.scalar.dma_start(out=brv_sb[:], in_=brv_bc)

            assert vdt == bf16 and sdt == bf16 and odt == bf16
            v4 = v_sb[:].rearrange("p (m h c) -> p m h c", m=NM, h=8, c=128)
            rv4 = rv_sb[:].rearrange("p (m h c) -> p m h c", m=NM, h=8, c=128)
            nc.vector.memset(v4[:, :, :, 64:128], 1.0)
            nc.vector.memset(rv4[:, :, :, 64:128], 1.0)

            # Score/exp pools opened BEFORE the projection pools so their PSUM
            # banks are disjoint from the projection psum banks (no false WAR:
            # scores may start as soon as q/k/rk chunks are ready).
            spool = ctx.enter_context(tc.tile_pool(name="spool", bufs=2, space="PSUM"))
            ppool = ctx.enter_context(tc.tile_pool(name="ppool", bufs=34))

            p_tiles = {}

            def emit_scores(lqh, dcs=(0, 1, 2, 3), brs=(0, 1)):
                for dc in dcs:
                    qsl = slice(1024 * dc + 512 * lqh, 1024 * dc + 512 * lqh + 512)
                    for m in range(NM):
                        ksl = slice(LKP * dc + 128 * m, LKP * dc + 128 * m + 128)
                        for br, kt in ((0, kT_sb), (1, rkT_sb)):
                            if br not in brs:
                                continue
                            s = spool.tile([128, 1024], f32, tag="spool", name="s")
                            nc.tensor.matmul(
                                s[:, 0:512], mm_in(kt[0:64, ksl]), mm_in(qT_sb[0:64, qsl])
                            )
                            nc.tensor.matmul(
                                s[:, 512:1024],
                                mm_in(kt[64:128, ksl]),
                                mm_in(qT_sb[64:128, qsl]),
                            )
                            p = ppool.tile([128, 1024], vdt, tag="ppool", name="p")
                            nc.scalar.activation(
                                prod(p[:]),
                                s[:],
                                Exp,
                                bias=maskb_sb[:, m : m + 1],
                                scale=SCALE,
                            )
                            p_tiles[(lqh, dc, m, br)] = p


            # ---------------- Phase 1: projections ----------------
            with ExitStack() as ph1:
                inp = ph1.enter_context(tc.tile_pool(name="inp", bufs=32))
                wch_pool = ph1.enter_context(tc.tile_pool(name="wch", bufs=24))
                ppsum = ph1.enter_context(
                    tc.tile_pool(name="ppsum", bufs=2, space="PSUM")
                )

                # Transposed projections: out chunk dc = lhsT(W block).T @ x_chunk.
                # The qk-branch scores are emitted between the k and rk
                # projections - they only need q+k, so the ACT exp stream
                # starts a full projection earlier.
                rk_xch = None
                proj_specs = [
                    ("k", xkT, wkT, bk_sb, kT_sb, sdt, LKP),
                    ("rk", xrT, wrkT, brk_sb, rkT_sb, vdt, LKP),
                ]
                for name, xt, wt, b_sb, out_sb, dt_, LL in proj_specs:
                    nsl = [slice(a, min(a + 512, LL)) for a in range(0, LL, 512)]
                    xch = []
                    wch = []
                    for k in range(8):
                        t = inp.tile([128, LL], dt_, tag="inp", name=f"x{name}{k}")
                        nc.sync.dma_start(
                            out=prod(t[:]), in_=mm_in(xt[128 * k : 128 * k + 128, :])
                        )
                        xch.append(t)
                        w = wch_pool.tile([128, HD], dt_, tag="wch", name=f"w{name}{k}")
                        nc.gpsimd.dma_start(
                            out=prod(w[:]), in_=mm_in(wt[128 * k : 128 * k + 128, :])
                        )
                        wch.append(w)
                    if name == "rk":
                        rk_xch = xch
                    # Staggered dc chains: chain dc's k4-7 matmuls interleave
                    # with chain dc+1's k0-3, so each input chunk is consumed
                    # at half rate and the PE never outruns the DMA supply at
                    # kernel start; bias-adds overlap the next stage.
                    def pmm(ps, dc, k):
                        for sl in nsl:
                            nc.tensor.matmul(
                                ps[:, sl],
                                mm_in(wch[k][:, 128 * dc : 128 * dc + 128]),
                                mm_in(xch[k][:, sl]),
                                start=(k == 0),
                                stop=(k == 7),
                            )

                    chains = [None] * 5
                    chains[0] = ppsum.tile(
                        [128, LL], f32, tag="ppsum", name=f"pp{name}0"
                    )
                    for k in range(4):
                        pmm(chains[0], 0, k)
                    for j in range(4):
                        if j + 1 < 4:
                            chains[j + 1] = ppsum.tile(
                                [128, LL], f32, tag="ppsum", name=f"pp{name}{j + 1}"
                            )
                        for k in range(4):
                            pmm(chains[j], j, 4 + k)
                            if j + 1 < 4:
                                pmm(chains[j + 1], j + 1, k)
                        nc.vector.tensor_scalar(
                            out=prod(out_sb[:, LL * j : LL * j + LL]),
                            in0=chains[j][:],
                            scalar1=b_sb[:, j : j + 1],
                            scalar2=None,
                            op0=Add,
                        )

                # q projection AFTER k/rk, one dc chain at a time with that
                # dc's lq-half-0 scores emitted immediately after: the first
                # exp fires right after k+rk+one q chain instead of after all
                # three projections (~20us earlier), and the k/rk PE time
                # covers the xq DMA supply so the chains never starve.
                xqch = []
                wqch = []
                for k in range(8):
                    t = inp.tile([128, LQ], sdt, tag="inp", name=f"xq{k}")
                    nc.sync.dma_start(
                        out=t[:], in_=xqT[128 * k : 128 * k + 128, :]
                    )
                    xqch.append(t)
                    w = wch_pool.tile([128, HD], sdt, tag="wch", name=f"wq{k}")
                    nc.gpsimd.dma_start(
                        out=w[:], in_=wqT[128 * k : 128 * k + 128, :]
                    )
                    wqch.append(w)
                for dc in range(4):
                    ps = ppsum.tile([128, LQ], f32, tag="ppsum", name=f"ppq{dc}")
                    for k in range(8):
                        for sl in (slice(0, 512), slice(512, 1024)):
                            nc.tensor.matmul(
                                ps[:, sl],
                                wqch[k][:, 128 * dc : 128 * dc + 128],
                                xqch[k][:, sl],
                                start=(k == 0),
                                stop=(k == 7),
                            )
                    nc.vector.tensor_scalar(
                        out=qT_sb[:, LQ * dc : LQ * dc + LQ],
                        in0=ps[:],
                        scalar1=bq_sb[:, dc : dc + 1],
                        scalar2=None,
                        op0=Add,
                    )
                    emit_scores(0, [dc])

                # Natural-orientation projections for v / rv. rv reuses rk's
                # input tiles (same weak_rela data, still resident in the inp
                # ring), and the second lq-half scores are split around rv so
                # the ACT engine never starves.
                def proj_N(name, xt, wt, b_sb, out4, xch_reuse):
                    if xch_reuse is not None:
                        xch = xch_reuse
                    else:
                        xch = []
                        for k in range(8):
                            t = inp.tile(
                                [128, LKP], vdt, tag="inp", name=f"x{name}{k}"
                            )
                            nc.sync.dma_start(
                                out=prod(t[:]),
                                in_=mm_in(xt[128 * k : 128 * k + 128, :]),
                            )
                            xch.append(t)
                    wch = []
                    for k in range(8):
                        w = wch_pool.tile([128, HD], vdt, tag="wch", name=f"w{name}{k}")
                        nc.gpsimd.dma_start(
                            out=prod(w[:]), in_=mm_in(wt[128 * k : 128 * k + 128, :])
                        )
                        wch.append(w)
                    for m in range(NM):
                        ps = ppsum.tile([128, 512], f32, tag="ppsum")
                        for k in range(8):
                            nc.tensor.matmul(
                                ps[:],
                                mm_in(xch[k][:, 128 * m : 128 * m + 128]),
                                mm_in(wch[k][:]),
                                start=(k == 0),
                                stop=(k == 7),
                            )
                        nc.vector.tensor_tensor(
                            out=prod(out4[:, m, :, 0:64]),
                            in0=ps[:].rearrange("p (h c) -> p h c", h=8, c=64),
                            in1=b_sb[:].rearrange("p (h c) -> p h c", h=8, c=64),
                            op=Add,
                        )

                proj_N("v", xvT, wvT, bv_sb, v4, None)
                emit_scores(1, [0, 1])
                proj_N("rv", xrT, wrvT, brv_sb, rv4, rk_xch)
                emit_scores(1, [2, 3])

            # -------- Phase B: PV accumulation, normalize, output projection ----
            with ExitStack() as ph2:
                xpool = ph2.enter_context(
                    tc.tile_pool(name="xpool", bufs=4, space="PSUM")
                )
                # One merged SBUF pool for all phase-2 scratch (tags keep
                # independent rings) - TileContext exit emits per-pool
                # per-engine semaphore teardown chains, so fewer pools means
                # a shorter serial teardown.
                p2sb = ph2.enter_context(tc.tile_pool(name="p2sb", bufs=8))
                xsb = sgp = bcp = rbp = wop = ysb = p2sb

                woch = []
                for dc in range(4):
                    w = wop.tile([128, 1024], odt, tag="wop", name=f"wo{dc}")
                    nc.sync.dma_start(
                        out=prod(w[:]), in_=mm_in(woT[128 * dc : 128 * dc + 128, :])
                    )
                    woch.append(w)

                def outproj_pair(lqh, pair, act=False):
                    # Two ot chains dc-outer (consecutive matmuls alternate
                    # banks), drained the moment they stop. Early pairs copy
                    # on DVE (ACT is streaming exps); late pairs copy on ACT
                    # (DVE is running the tail normalize).
                    ps2 = {
                        ot: xpool.tile(
                            [128, 512], f32, tag="xpool", name=f"psy{ot}"
                        )
                        for ot in pair
                    }
                    for dc in range(4):
                        for ot in pair:
                            nc.tensor.matmul(
                                ps2[ot][:],
                                mm_in(woch[dc][:, 128 * ot : 128 * ot + 128]),
                                mm_in(
                                    xf_sb[
                                        :,
                                        1024 * dc
                                        + 512 * lqh : 1024 * dc
                                        + 512 * lqh
                                        + 512,
                                    ]
                                ),
                                start=(dc == 0),
                                stop=(dc == 3),
                            )
                    for ot in pair:
                        y = ysb.tile([128, 512], odt, tag="ysb")
                        if act:
                            nc.scalar.activation(y[:], ps2[ot][:], Copy)
                        else:
                            nc.vector.tensor_copy(out=y[:], in_=ps2[ot][:])
                        (nc.sync if ot % 2 == 0 else nc.gpsimd).dma_start(
                            out=yT[
                                128 * ot : 128 * ot + 128,
                                512 * lqh : 512 * lqh + 512,
                            ],
                            in_=y[:],
                        )

                def emit_it(lqh, dc, tail=False):
                    xacc = {}
                    accs = [(0, 0), (1, 0), (0, 1), (1, 1)]
                    for br, hs in accs:
                        xacc[(br, hs)] = xpool.tile(
                            [128, 512], f32, tag="xpool", name=f"xacc{br}{hs}"
                        )
                    vvs = {0: v4, 1: rv4}

                    def pv_step(br, hs, m):
                        pt = p_tiles[(lqh, dc, m, br)]
                        nc.tensor.matmul(
                            xacc[(br, hs)][:],
                            mm_in(vvs[br][:, m, 2 * dc + hs, :]),
                            mm_in(pt[:, 512 * hs : 512 * hs + 512]),
                            start=(m == 0),
                            stop=(m == NM - 1),
                        )

                    xf_slice = slice(
                        1024 * dc + 512 * lqh, 1024 * dc + 512 * lqh + 512
                    )
                    if tail:
                        # hs=0 chains first so their normalize overlaps the
                        # hs=1 PV matmuls; consecutive matmuls still alternate
                        # banks within each pair.
                        for hs in range(2):
                            for m in range(NM):
                                for br in range(2):
                                    pv_step(br, hs, m)
                            ts = {}
                            for br in range(2):
                                zt = rbp.tile([64, 512], f32, tag="zt")
                                nc.scalar.activation(
                                    zt[:], xacc[(br, hs)][64:128, :], Copy
                                )
                                rb = rbp.tile([64, 512], f32, tag="rbp")
                                nc.vector.reciprocal_approx_fast(
                                    out=rb[:], in_=zt[:]
                                )
                                tt = xsb.tile([65, 512], vdt, tag="xsb")
                                nc.vector.tensor_tensor(
                                    out=prod(tt[0:64, :]),
                                    in0=xacc[(br, hs)][0:64, :],
                                    in1=rb[:],
                                    op=Mult,
                                )
                                ts[br] = tt
                            nc.vector.tensor_tensor(
                                out=prod(xf_sb[64 * hs : 64 * hs + 64, xf_slice]),
                                in0=ts[0][0:64, :],
                                in1=ts[1][0:64, :],
                                op=Add,
                            )
                        return
                    for m in range(NM):
                        for br, hs in accs:
                            pv_step(br, hs, m)
                    # Copy x rows + denominator row 64 to SBUF (frees the PSUM
                    # banks early), packed so the 4 denominator rows ship to
                    # DRAM in a single DMA.
                    xs_all = xsb.tile([65, 4 * 512], vdt, tag="xsall", bufs=3)
                    xs = {}
                    for j, (br, hs) in enumerate(accs):
                        sl = xs_all[:, 512 * j : 512 * j + 512]
                        nc.vector.tensor_copy(out=sl, in_=xacc[(br, hs)][0:65, :])
                        xs[(br, hs)] = sl
                    # Batch-reciprocate the 4 denominator rows: direct
                    # SBUF->SBUF reshape onto 128 partitions, reciprocal,
                    # one DRAM bounce for the partition broadcast.
                    it = 2 * dc + lqh
                    sg = sgp.tile([128, 16], vdt, tag="sgp")
                    nc.sync.dma_start(out=sg[:], in_=xs_all[64:65, :])
                    nc.vector.reciprocal(sg[:], sg[:])
                    nc.sync.dma_start(out=scr2[it, :], in_=sg[:])
                    for hs in range(2):
                        jv, jr = 2 * hs, 2 * hs + 1
                        bcv = bcp.tile([64, 512], vdt, tag="bcp", name="bcv")
                        nc.gpsimd.dma_start(
                            out=bcv[:],
                            in_=scr2[it : it + 1, 512 * jv : 512 * jv + 512]
                            .partition_broadcast(64)[:, 0, :],
                        )
                        bcr = bcp.tile([64, 512], vdt, tag="bcp", name="bcr")
                        nc.gpsimd.dma_start(
                            out=bcr[:],
                            in_=scr2[it : it + 1, 512 * jr : 512 * jr + 512]
                            .partition_broadcast(64)[:, 0, :],
                        )
                        t1 = xsb.tile([65, 512], vdt, tag="xsb")
                        nc.vector.tensor_tensor(
                            out=t1[0:64, :],
                            in0=xs[(0, hs)][0:64, :],
                            in1=bcv[:],
                            op=Mult,
                        )
                        t2 = xsb.tile([65, 512], vdt, tag="xsb")
                        nc.vector.tensor_tensor(
                            out=t2[0:64, :],
                            in0=xs[(1, hs)][0:64, :],
                            in1=bcr[:],
                            op=Mult,
                        )
                        nc.vector.tensor_tensor(
                            out=prod(xf_sb[64 * hs : 64 * hs + 64, xf_slice]),
                            in0=t1[0:64, :],
                            in1=t2[0:64, :],
                            op=Add,
                        )
                emit_it(0, 0)
                emit_it(0, 1)
                emit_it(0, 2)
                emit_it(0, 3)
                emit_it(1, 0)
                emit_it(1, 1, tail=True)
                # lq-half-0 output projection, wide: all 8 ot accumulators
                # live at once (4 xpool banks + 2 spool tiles split in half),
                # dc-outer so consecutive matmuls alternate banks.
                pssw = []
                for i in range(4):
                    pssw.append(
                        xpool.tile([128, 512], f32, tag="xpool", name=f"psw{i}")
                    )
                for i in range(2):
                    w2 = spool.tile([128, 1024], f32, tag="spool", name=f"psw2{i}")
                    pssw.append(w2[:, 0:512])
                    pssw.append(w2[:, 512:1024])
                for dc in range(4):
                    for ot in range(8):
                        nc.tensor.matmul(
                            pssw[ot],
                            mm_in(woch[dc][:, 128 * ot : 128 * ot + 128]),
                            mm_in(xf_sb[:, 1024 * dc : 1024 * dc + 512]),
                            start=(dc == 0),
                            stop=(dc == 3),
                        )
                for ot in range(8):
                    y = ysb.tile([128, 512], odt, tag="ysb")
                    if ot % 2 == 0:
                        nc.scalar.activation(y[:], pssw[ot], Copy)
                    else:
                        nc.vector.tensor_copy(out=y[:], in_=pssw[ot])
                    (nc.sync if ot % 2 == 0 else nc.gpsimd).dma_start(
                        out=yT[128 * ot : 128 * ot + 128, 0:512],
                        in_=y[:],
                    )
                emit_it(1, 2, tail=True)
                # Pre-emit ot0-3 of the lq-half-1 output projection over the
                # first three dim chunks on the spool banks (free once scores
                # are done), so those 12 matmuls overlap the final dance and
                # only ot0-3's dc3 plus ot4-7 trail the last normalize.
                pss = []
                for i in range(2):
                    w2 = spool.tile([128, 1024], f32, tag="spool", name=f"psw2{i}")
                    pss.append(w2[:, 0:512])
                    pss.append(w2[:, 512:1024])
                for dcc in range(3):
                    for ot in range(4):
                        nc.tensor.matmul(
                            pss[ot],
                            mm_in(woch[dcc][:, 128 * ot : 128 * ot + 128]),
                            mm_in(xf_sb[:, 1024 * dcc + 512 : 1024 * dcc + 1024]),
                            start=(dcc == 0),
                            stop=False,
                        )
                emit_it(1, 3, tail=True)

                # Drain an ot chain the moment it stops: copy alternates
                # ACT/DVE, stores go on the sync/scalar queues (gpsimd stays
                # empty at the end so its completion drain is off the
                # critical path).
                def drain_ot(ot):
                    y = ysb.tile([128, 512], odt, tag="ysb")
                    if ot % 2 == 0:
                        nc.scalar.activation(y[:], pss[ot], Copy)
                    else:
                        nc.vector.tensor_copy(out=y[:], in_=pss[ot])
                    nc.sync.dma_start(
                        out=yT[128 * ot : 128 * ot + 128, 512:1024], in_=y[:]
                    )

                for ot in range(4):
                    nc.tensor.matmul(
                        pss[ot],
                        mm_in(woch[3][:, 128 * ot : 128 * ot + 128]),
                        mm_in(xf_sb[:, 1024 * 3 + 512 : 1024 * 3 + 1024]),
                        start=False,
                        stop=True,
                    )
                    drain_ot(ot)
                for i in range(4):
                    pss.append(
                        xpool.tile([128, 512], f32, tag="xpool", name=f"psw{i}")
                    )
                # ot4-7 in two pairs: the first pair's copies and stores
                # overlap the second pair's matmuls.
                for pair in ((4, 5), (6, 7)):
                    for dcc in range(4):
                        for ot in pair:
                            nc.tensor.matmul(
                                pss[ot],
                                mm_in(woch[dcc][:, 128 * ot : 128 * ot + 128]),
                                mm_in(
                                    xf_sb[:, 1024 * dcc + 512 : 1024 * dcc + 1024]
                                ),
                                start=(dcc == 0),
                                stop=(dcc == 3),
                            )
                    for ot in pair:
                        drain_ot(ot)

    nc.compile()
    return nc


def _get_program(lkp=LKP):
    if lkp not in _CACHE:
        _CACHE[lkp] = _build_program(lkp)
    return _CACHE[lkp]


def _f8c(arr):
    """[1024, L] fp32 -> [128, 8*L] fp8e4m3 chunked: [p, k, n] = arr[128k+p, n]."""
    import ml_dtypes

    a = np.ascontiguousarray(arr, dtype=np.float32)
    L = a.shape[1]
    return np.ascontiguousarray(
        a.reshape(8, 128, L)
        .transpose(1, 0, 2)
        .reshape(128, 8 * L)
        .astype(ml_dtypes.float8_e4m3)
    )


def _cast(arr, bf16_flag):
    if bf16_flag:
        import ml_dtypes

        return np.ascontiguousarray(arr.astype(ml_dtypes.bfloat16))
    return np.ascontiguousarray(arr.astype(np.float32))


def _shard_inputs(inputs, lkp=LKP):
    q = np.ascontiguousarray(inputs["query"], dtype=np.float32)
    k = np.ascontiguousarray(inputs["key"], dtype=np.float32)
    v = np.ascontiguousarray(inputs["value"], dtype=np.float32)
    wr = np.ascontiguousarray(inputs["weak_rela"], dtype=np.float32)
    mask = np.asarray(inputs["mask"])
    sb, vb, ob = CFG["score_bf16"], CFG["v_bf16"], CFG["out_bf16"]

    in_maps = []
    for c in range(N_CORES):
        b, hh = divmod(c, 2)
        hsl = slice(HD * hh, HD * hh + HD)
        idx = np.nonzero(mask[b, 0])[0]
        nv = len(idx)
        assert nv <= lkp
        pidx = np.concatenate([idx, np.zeros(lkp - nv, dtype=idx.dtype)])
        bias = np.full(lkp, -1.0e9, np.float32)
        bias[:nv] = 0.0
        mb = np.ascontiguousarray(bias.reshape(lkp // 128, 128).T)
        kc, vc, wrc = k[b][pidx], v[b][pidx], wr[b][pidx]
        m = {
            "xqT": _cast(q[b].T, sb),
            "xkT": _cast(kc.T, sb),
            "xrT": _cast(wrc.T, vb),
            "xvT": _cast(vc.T, vb),
            "wqT": _cast(np.asarray(inputs["Wq"])[hsl, :].T, sb),
            "wkT": _cast(np.asarray(inputs["Wk"])[hsl, :].T, sb),
            "wrkT": _cast(np.asarray(inputs["Wrk"])[hsl, :].T, vb),
            "wvT": _cast(np.asarray(inputs["Wv"])[hsl, :].T, vb),
            "wrvT": _cast(np.asarray(inputs["Wrv"])[hsl, :].T, vb),
            "woT": _cast(np.asarray(inputs["Wo"])[:, hsl].T, ob),
            "bq_pc": np.asarray(inputs["bq"][hsl])
            .reshape(4, 128)
            .T.astype(np.float32),
            "bk_pc": np.asarray(inputs["bk"][hsl])
            .reshape(4, 128)
            .T.astype(np.float32),
            "brk_pc": np.asarray(inputs["brk"][hsl])
            .reshape(4, 128)
            .T.astype(np.float32),
            "bv_bc": np.broadcast_to(inputs["bv"][hsl], (128, HD)).astype(np.float32),
            "brv_bc": np.broadcast_to(inputs["brv"][hsl], (128, HD)).astype(
                np.float32
            ),
            "maskb": mb,
        }
        in_maps.append({k2: np.ascontiguousarray(v2) for k2, v2 in m.items()})
    return in_maps


def run_on_hw(inputs, trace=False, **kw):
    from concourse.bass_utils import run_bass_kernel_spmd

    mask = np.asarray(inputs["mask"])
    max_valid = max(int(mask[b, 0].sum()) for b in range(B))
    lkp = max(LKP, ((max_valid + 127) // 128) * 128)
    nc = _get_program(lkp)
    in_maps = _shard_inputs(inputs, lkp)
    res = run_bass_kernel_spmd(
        nc, in_maps, core_ids=list(range(N_CORES)), trace=trace, **kw
    )
    bo = np.asarray(inputs["bo"], dtype=np.float32)
    outs = []
    for b in range(B):
        yt = res.results[2 * b]["yT"].astype(np.float32) + res.results[
            2 * b + 1
        ]["yT"].astype(np.float32)
        outs.append(yt.T + bo)
    out = np.stack(outs).astype(np.float32)
    return out, res


def kernel(**inputs):
    out, _ = run_on_hw(inputs)
    return out

